# revision 27
# baseline (speedup 1.0000x reference)
"""SwiGLU expert FFN (DbrxExpertGLU) on 8 Trainium2 NeuronCores.

reference: down = (silu(x @ w1) * (x @ v1)) @ w2.T
  x [T=4096, H=4096], w1/v1/w2 [H=4096, F=14336], all fp32.

Strategy: token-parallel — shard T across the 8 cores (512 rows each),
replicate weights; no collectives. Each core computes everything
transposed so both matmul stages contract over the partition dim:

  phase 1:  gateT/upT [F, m] = w1T/v1T-tiles.T @ xT   (accumulate over H)
            hT = silu(gateT) * upT                     (elementwise, bf16)
  phase 2:  downT [H, m]     = w2T-tiles.T @ hT        (accumulate over F)

Matmuls run in bf16 (PE peak 78.6 TF/s) with fp32 PSUM accumulation.
Host pre-casts weights to bf16 and retiles them so every weight DMA is
a contiguous [128, *] full-partition transfer.

Schedule notes (from neuron-profile trace analysis):
  - PE cadence is 216 ns per [128x128]x[128,512] matmul at 2.4 GHz
    (259 ns when the chip's power throttle caps the PE at 2.0 GHz —
    a sticky, run-to-run state that explains ~1.2x exec-time swings),
    so the bf16 matmul floor is 2.29-2.78 ms/core depending on clock;
    everything else is overlap. fp8/int8 cannot beat this: DoubleRow
    is e4m3/e5m2-only and their quantization noise (3.8-6.6% end to
    end) fails the 2e-2 gate.
  - Warmup junk matmuls are NOT gated on any DMA (first data lands
    ~10-13 us after the body gate) so the HAM clock ramp and the DMA
    ramp run concurrently; the prefix defers the first real matmul to
    ~17.5 us (2.4 GHz) — every earlier start measured worse: real
    matmuls land in the cold 1.2 GHz window and the erratic early
    DMA ramp (0.2-0.42 MB/us) adds a 2-5 us contiguous stall that
    trips a HAM re-throttle.
  - gate/up matmuls interleave per ko; x streams in 2-ko/256KB chunks
    and w1/v1 in 8-ko/256KB chunks on the single Sync HWDGE ring in
    exact consumption order (a Sync+Scalar two-ring split delivers
    out of order and starves the x stream).
  - w2[hh=0] is prefetched into a persistent pool during phase 1
    (paced 2 chunks per ff iteration so it never starves the w1/v1
    ring), eliminating the 15 us phase-1->phase-2 DMA gap.
  - The last hh runs as two N=256 accumulation chains so half the
    final copy+DMA hides under matmuls (exposed tail 5.3 -> 2.5 us).

Set KERNEL_TRACE=1 to capture an NTFF profile; the HW exec time lands
in kernel.last_exec_time_ns.
"""

import os

import numpy as np
import ml_dtypes

import concourse.mybir as mybir
from concourse import bacc, bass_utils
from concourse.tile import TileContext

T, H, F = 4096, 4096, 14336
NCORES = 8
M = T // NCORES  # 512 token rows per core
P = 128
KO = H // P  # 32  k-tiles (phase-1 contraction)
FFO = F // P  # 112 f-tiles
HHO = H // P  # 32  output-row tiles (phase-2)
BF16 = mybir.dt.bfloat16
FP32 = mybir.dt.float32

N_JUNK_BIG = 18  # 512-row warmup matmuls (PE clock ramp, DMA spin-up)
N_JUNK_SMALL = 40  # 128-row warmup matmuls (fine-grained bridge)
W2PRE = 44  # ff-chunks of w2[hh=0] prefetched during phase 1
XCH = 16  # x startup chunks (2 ko / 256KB each)
WCH = 4  # w1/v1 ff=0 startup chunks (8 ko / 256KB each)

last_exec_time_ns = None
_cache = {}


def _build():
    nc = bacc.Bacc("TRN2", target_bir_lowering=False, debug=False)
    xT_d = nc.dram_tensor("xT", [P, KO * M], BF16, kind="ExternalInput").ap()
    w1_d = nc.dram_tensor("w1t", [FFO, P, KO * P], BF16, kind="ExternalInput").ap()
    v1_d = nc.dram_tensor("v1t", [FFO, P, KO * P], BF16, kind="ExternalInput").ap()
    w2_d = nc.dram_tensor("w2t", [HHO, P, FFO * P], BF16, kind="ExternalInput").ap()
    out_d = nc.dram_tensor("outT", [HHO, P, M], FP32, kind="ExternalOutput").ap()

    with TileContext(nc) as tc:
        with (
            tc.tile_pool(name="hpool", bufs=1) as hpool,
            tc.tile_pool(name="w2fpool", bufs=1) as w2fpool,
            # Opened before the phase-1 PSUM pools so its banks never
            # collide with them: the first phase-2 matmul must not wait
            # for the last phase-1 silu/mul PSUM reads (WAR).
            tc.tile_pool(name="ps2", bufs=2, space="PSUM") as ps2,
        ):
            # hT[ff] tiles live here across both phases:
            # slice [:, ff*M:(ff+1)*M] holds hT rows ff*128..ff*128+127.
            hT = hpool.tile([P, FFO * M], BF16)
            # First W2PRE ff-chunks of w2[hh=0], filled during phase 1.
            w2_first = w2fpool.tile([P, W2PRE * P], BF16)

            with (
                tc.tile_pool(name="xpool", bufs=1) as xpool,
                tc.tile_pool(name="wpool", bufs=3) as wpool,
                tc.tile_pool(name="pspool", bufs=2, space="PSUM") as pspool,
                tc.tile_pool(name="actpool", bufs=2) as actpool,
                tc.tile_pool(name="warmps", bufs=1, space="PSUM") as warmps,
            ):
                # Warm the PE HAM clock gate while the DMA engines spin up
                # and the initial tiles stream in. Inputs are uninitialized
                # SBUF garbage (this kernel only ever runs on hardware —
                # no sim uninit checks) so the junk has no dependencies at
                # all and starts as soon as the queue drains the preamble.
                # An idle PE down-clocks (~2x) for ~3us on resume, so junk
                # also pads the DMA-bound stretches below.
                warmp = warmps.tile([P, 4 * P], FP32)
                for i in range(N_JUNK_BIG):
                    nc.tensor.matmul(
                        warmp[:], hT[:, :P], hT[:, : 4 * P],
                        start=(i == 0), stop=(i == N_JUNK_BIG - 1),
                    )
                for i in range(N_JUNK_SMALL):
                    nc.tensor.matmul(
                        warmp[:, :P], hT[:, :P], hT[:, :P],
                        start=(i == 0), stop=(i == N_JUNK_SMALL - 1),
                    )

                def _junk():
                    nc.tensor.matmul(
                        warmp[:, :P], hT[:, :P], hT[:, :P],
                        start=True, stop=True,
                    )

                # xT resident: xt[ki, ko*M+m] = x[m, ko*128+ki]. The first
                # w1/v1 tiles are DMA'd per-ko interleaved with x so the
                # ff=0 matmuls can chase the DMA stream.
                # Descriptor issue on the sync engine costs ~0.6us each, so
                # startup chunks start small (first matmul deps land early)
                # and coarsen, issued in the order the interleaved gate/up
                # matmuls consume them.
                xt = xpool.tile([P, KO * M], BF16)
                w1t0 = wpool.tile([P, KO * P], BF16, tag="w1")
                v1t0 = wpool.tile([P, KO * P], BF16, tag="v1")
                w1t1 = wpool.tile([P, KO * P], BF16, tag="w1")
                v1t1 = wpool.tile([P, KO * P], BF16, tag="v1")

                # All startup DMAs go on the single Sync HWDGE ring, in
                # exact consumption order: the ring drains FIFO, so the
                # stream never spends bandwidth on bytes the PE doesn't
                # need yet. Chunks are <=256KB so a consumer never waits
                # on more than ~0.7us of in-flight transfer — the coarse
                # 512KB x chunks were what the slow-ramp runs' 2.5-4us
                # stalls (and the HAM re-throttle they tripped) waited
                # on. The junk prefix still defers the first real matmul
                # to ~17.5us: starting earlier ran real matmuls inside
                # the cold 1.2GHz HAM window and lost every time.
                XS = KO * M // XCH  # x chunk cols (2 ko)
                WS = KO * P // WCH  # w chunk cols (8 ko)

                def _dx(c):
                    nc.sync.dma_start(
                        out=xt[:, c * XS : (c + 1) * XS],
                        in_=xT_d[:, c * XS : (c + 1) * XS],
                    )

                def _dw(c):
                    nc.sync.dma_start(
                        out=w1t0[:, c * WS : (c + 1) * WS],
                        in_=w1_d[0][:, c * WS : (c + 1) * WS],
                    )
                    nc.sync.dma_start(
                        out=v1t0[:, c * WS : (c + 1) * WS],
                        in_=v1_d[0][:, c * WS : (c + 1) * WS],
                    )

                def _dw1(t, s, k0, k1):
                    nc.sync.dma_start(
                        out=t[:, k0 * P : k1 * P], in_=s[1][:, k0 * P : k1 * P]
                    )

                # x chunk c covers ko 2c..2c+1; w/v chunk c covers ko
                # 8c..8c+7. Issue each just before its consumers.
                _dx(0); _dw(0); _dx(1); _dx(2); _dx(3)
                _dx(4); _dw(1); _dx(5); _dx(6); _dx(7)
                _dx(8); _dw(2); _dx(9); _dx(10); _dx(11)
                _dx(12); _dw(3); _dx(13); _dx(14); _dx(15)
                # w1/v1 for ff=1 in halves: the first halves land ~2us
                # earlier than a single full-tile transfer, closing the
                # ff=0 -> ff=1 handoff stall.
                _dw1(w1t1, w1_d, 0, KO // 2)
                _dw1(v1t1, v1_d, 0, KO // 2)
                _dw1(w1t1, w1_d, KO // 2, KO)
                _dw1(v1t1, v1_d, KO // 2, KO)

                for ff in range(FFO):
                    # w tile: [ki, ko*128+f] = w1[ko*128+ki, ff*128+f]
                    if ff == 0:
                        w1_tile, v1_tile = w1t0, v1t0
                    elif ff == 1:
                        w1_tile, v1_tile = w1t1, v1t1
                    else:
                        w1_tile = wpool.tile([P, KO * P], BF16, tag="w1")
                        v1_tile = wpool.tile([P, KO * P], BF16, tag="v1")
                        nc.sync.dma_start(out=w1_tile[:], in_=w1_d[ff])
                        nc.sync.dma_start(out=v1_tile[:], in_=v1_d[ff])
                        # Pace the w2[0] prefetch behind this iteration's
                        # weight fetches: 8 ff-chunks per iteration.
                        c = ff - 4
                        lo, hi = c * 8 * P, min((c + 1) * 8, W2PRE) * P
                        if 0 <= c and lo < W2PRE * P:
                            nc.sync.dma_start(
                                out=w2_first[:, lo:hi],
                                in_=w2_d[0][:, lo:hi],
                            )

                    pg = pspool.tile([P, M], FP32, tag="pg")
                    pu = pspool.tile([P, M], FP32, tag="pu")
                    for ko in range(KO):
                        nc.tensor.matmul(
                            pg[:],
                            w1_tile[:, ko * P : (ko + 1) * P],
                            xt[:, ko * M : (ko + 1) * M],
                            start=(ko == 0),
                            stop=(ko == KO - 1),
                        )
                        nc.tensor.matmul(
                            pu[:],
                            v1_tile[:, ko * P : (ko + 1) * P],
                            xt[:, ko * M : (ko + 1) * M],
                            start=(ko == 0),
                            stop=(ko == KO - 1),
                        )
                        # A little junk through the first kos pads the
                        # earliest chunk raggedness. Beyond that it's
                        # useless in both regimes (trace-verified): on
                        # delivery-bound 2.4GHz runs the stalls happen
                        # with or without it, and on PE-paced 2.0GHz runs
                        # each junk adds its full 53-64ns to ff=0.
                        if ff == 0 and ko < 8:
                            _junk()
                    sg = actpool.tile([P, M], FP32, tag="sg")
                    nc.scalar.activation(
                        sg[:], pg[:], mybir.ActivationFunctionType.Silu
                    )
                    nc.vector.tensor_mul(
                        out=hT[:, ff * M : (ff + 1) * M], in0=sg[:], in1=pu[:]
                    )

            with (
                tc.tile_pool(name="w2tailp", bufs=1) as w2tailp,
                tc.tile_pool(name="w2pool", bufs=2) as w2pool,
                tc.tile_pool(name="opool", bufs=2) as opool,
            ):
                # Chunks of w2[hh=0] that didn't fit next to the phase-1
                # pools; fetched first thing in phase 2. Split into 4
                # descriptors so the first chunks' completion doesn't
                # wait on the whole transfer (hh=0 consumes chunk c at
                # ~c*216ns into phase 2).
                NT = FFO - W2PRE
                w2_tail = w2tailp.tile([P, NT * P], BF16)
                for t0 in range(0, NT, (NT + 3) // 4):
                    t1 = min(t0 + (NT + 3) // 4, NT)
                    nc.sync.dma_start(
                        out=w2_tail[:, t0 * P : t1 * P],
                        in_=w2_d[0][:, (W2PRE + t0) * P : (W2PRE + t1) * P],
                    )
                half = M // 2
                for hh in range(HHO):
                    # w2 tile: [ki, ffo*128+f] = w2[hh*128+f, ffo*128+ki]
                    if hh == 0:
                        w2_tile = None
                    else:
                        w2_tile = w2pool.tile([P, FFO * P], BF16, tag="w2")
                        nc.sync.dma_start(out=w2_tile[:], in_=w2_d[hh])

                    def _w2src(ff):
                        if hh == 0:
                            if ff < W2PRE:
                                return w2_first[:, ff * P : (ff + 1) * P]
                            c = ff - W2PRE
                            return w2_tail[:, c * P : (c + 1) * P]
                        return w2_tile[:, ff * P : (ff + 1) * P]

                    if hh < HHO - 1:
                        pd = ps2.tile([P, M], FP32, tag="pd")
                        for ff in range(FFO):
                            nc.tensor.matmul(
                                pd[:],
                                _w2src(ff),
                                hT[:, ff * M : (ff + 1) * M],
                                start=(ff == 0),
                                stop=(ff == FFO - 1),
                            )
                        ot = opool.tile([P, M], FP32, tag="ot")
                        nc.vector.tensor_copy(out=ot[:], in_=pd[:])
                        nc.sync.dma_start(out=out_d[hh], in_=ot[:])
                    else:
                        # Last tile: two half-width accumulation chains so
                        # the first half's copy+DMA hides under the second
                        # half's matmuls; only ~1.9us of copy+DMA remains
                        # exposed after the final matmul. Full-size [P, M]
                        # psum tiles keep the two chains in separate banks
                        # (PE-write + DVE-read same bank is fatal).
                        for c in range(2):
                            lo = c * half
                            pd = ps2.tile([P, M], FP32, tag="pd")
                            for ff in range(FFO):
                                nc.tensor.matmul(
                                    pd[:, :half],
                                    _w2src(ff),
                                    hT[:, ff * M + lo : ff * M + lo + half],
                                    start=(ff == 0),
                                    stop=(ff == FFO - 1),
                                )
                            ot = opool.tile([P, M], FP32, tag="ot")
                            nc.vector.tensor_copy(
                                out=ot[:, :half], in_=pd[:, :half]
                            )
                            nc.sync.dma_start(
                                out=out_d[hh][:, lo : lo + half],
                                in_=ot[:, :half],
                            )
    nc.compile()
    return nc


def _prep_weights(expert_w1, expert_v1, expert_w2):
    bf = ml_dtypes.bfloat16
    # w1t[ffo, ki, ko*P+f] = w1[ko*P+ki, ffo*P+f]
    w1t = np.ascontiguousarray(
        expert_w1.reshape(KO, P, FFO, P).transpose(2, 1, 0, 3).reshape(FFO, P, KO * P)
    ).astype(bf)
    v1t = np.ascontiguousarray(
        expert_v1.reshape(KO, P, FFO, P).transpose(2, 1, 0, 3).reshape(FFO, P, KO * P)
    ).astype(bf)
    # w2t[hho, ki, ffo*P+f] = w2[hho*P+f, ffo*P+ki]
    w2t = np.ascontiguousarray(
        expert_w2.reshape(HHO, P, FFO, P).transpose(0, 3, 2, 1).reshape(HHO, P, FFO * P)
    ).astype(bf)
    return w1t, v1t, w2t


def kernel(x, expert_w1, expert_v1, expert_w2):
    global last_exec_time_ns
    x = np.asarray(x, dtype=np.float32)
    w1t, v1t, w2t = _prep_weights(
        np.asarray(expert_w1, np.float32),
        np.asarray(expert_v1, np.float32),
        np.asarray(expert_w2, np.float32),
    )

    bf = ml_dtypes.bfloat16
    in_maps = []
    for c in range(NCORES):
        xs = x[c * M : (c + 1) * M]  # [M, H]
        # xt[ki, ko*M+m] = xs[m, ko*P+ki]
        xt = np.ascontiguousarray(
            xs.reshape(M, KO, P).transpose(2, 1, 0).reshape(P, KO * M)
        ).astype(bf)
        in_maps.append({"xT": xt, "w1t": w1t, "v1t": v1t, "w2t": w2t})

    if "nc" not in _cache:
        _cache["nc"] = _build()
    nc = _cache["nc"]

    trace = os.environ.get("KERNEL_TRACE", "") == "1"
    if trace:
        _install_ntff_hook()
    res = None
    for attempt in range(3):
        try:
            res = bass_utils.run_bass_kernel_spmd(
                nc, in_maps, core_ids=list(range(NCORES)), trace=trace
            )
            break
        except Exception:
            # The tunneled device occasionally reports a transient
            # "unrecoverable" state left over from a prior session; it
            # clears on retry.
            if attempt == 2:
                raise
            import time

            time.sleep(20)
    last_exec_time_ns = res.exec_time_ns

    # results[c]["outT"] is downT for core c: [HHO, P, M] with
    # outT[hh, j, m] = down[c*M+m, hh*P+j]
    out = np.empty((T, H), np.float32)
    for c in range(NCORES):
        o = res.results[c]["outT"].reshape(H, M)
        out[c * M : (c + 1) * M] = o.T
    return out


def _install_ntff_hook():
    """Wire the axon NTFF profile hook this image's antenv lacks."""
    import importlib.util
    import sys
    import types

    if "antenv.axon_hooks" in sys.modules:
        return
    so_path = "/opt/axon/libaxon_pjrt.so"
    boot = "/root/.axon_site/trn_agent_boot/trn_boot.py"
    if not (os.path.exists(so_path) and os.path.exists(boot)):
        return
    spec = importlib.util.spec_from_file_location("trn_boot_local", boot)
    trn_boot = importlib.util.module_from_spec(spec)
    spec.loader.exec_module(trn_boot)
    hook = trn_boot._ntff_profile_via_ctypes(so_path)
    m = types.ModuleType("antenv.axon_hooks")
    m.get_axon_ntff_profile_hook = lambda: hook
    m.set_axon_ntff_profile_hook = lambda h: None
    sys.modules["antenv.axon_hooks"] = m



# revision 28
# speedup vs baseline: 1.0026x; 1.0026x over previous
"""SwiGLU expert FFN (DbrxExpertGLU) on 8 Trainium2 NeuronCores.

reference: down = (silu(x @ w1) * (x @ v1)) @ w2.T
  x [T=4096, H=4096], w1/v1/w2 [H=4096, F=14336], all fp32.

Strategy: token-parallel — shard T across the 8 cores (512 rows each),
replicate weights; no collectives. Each core computes everything
transposed so both matmul stages contract over the partition dim:

  phase 1:  gateT/upT [F, m] = w1T/v1T-tiles.T @ xT   (accumulate over H)
            hT = silu(gateT) * upT                     (elementwise, bf16)
  phase 2:  downT [H, m]     = w2T-tiles.T @ hT        (accumulate over F)

Matmuls run in bf16 (PE peak 78.6 TF/s) with fp32 PSUM accumulation.
Host pre-casts weights to bf16 and retiles them so every weight DMA is
a contiguous [128, *] full-partition transfer.

Schedule notes (from neuron-profile trace analysis):
  - PE cadence is 216 ns per [128x128]x[128,512] matmul at 2.4 GHz
    (259 ns when the chip's power throttle caps the PE at 2.0 GHz —
    a sticky, run-to-run state that explains ~1.2x exec-time swings),
    so the bf16 matmul floor is 2.29-2.78 ms/core depending on clock;
    everything else is overlap. fp8/int8 cannot beat this: DoubleRow
    is e4m3/e5m2-only and their quantization noise (3.8-6.6% end to
    end) fails the 2e-2 gate.
  - Warmup junk matmuls are NOT gated on any DMA (first data lands
    ~10-13 us after the body gate) so the HAM clock ramp and the DMA
    ramp run concurrently; the prefix defers the first real matmul to
    ~17.5 us (2.4 GHz) — every earlier start measured worse: real
    matmuls land in the cold 1.2 GHz window and the erratic early
    DMA ramp (0.2-0.42 MB/us) adds a 2-5 us contiguous stall that
    trips a HAM re-throttle.
  - gate/up matmuls interleave per ko; x streams in 2-ko/256KB chunks
    and w1/v1 in 8-ko/256KB chunks on the single Sync HWDGE ring in
    exact consumption order (a Sync+Scalar two-ring split delivers
    out of order and starves the x stream).
  - w2[hh=0] is prefetched into a persistent pool during phase 1
    (paced 2 chunks per ff iteration so it never starves the w1/v1
    ring), eliminating the 15 us phase-1->phase-2 DMA gap.
  - The last hh runs as two N=256 accumulation chains so half the
    final copy+DMA hides under matmuls (exposed tail 5.3 -> 2.5 us).

Set KERNEL_TRACE=1 to capture an NTFF profile; the HW exec time lands
in kernel.last_exec_time_ns.
"""

import os

import numpy as np
import ml_dtypes

import concourse.mybir as mybir
from concourse import bacc, bass_utils
from concourse.tile import TileContext

T, H, F = 4096, 4096, 14336
NCORES = 8
M = T // NCORES  # 512 token rows per core
P = 128
KO = H // P  # 32  k-tiles (phase-1 contraction)
FFO = F // P  # 112 f-tiles
HHO = H // P  # 32  output-row tiles (phase-2)
BF16 = mybir.dt.bfloat16
FP32 = mybir.dt.float32

N_JUNK_BIG = 18  # 512-row warmup matmuls (PE clock ramp, DMA spin-up)
N_JUNK_SMALL = 40  # 128-row warmup matmuls (fine-grained bridge)
W2PRE = 44  # ff-chunks of w2[hh=0] prefetched during phase 1
XCH = 16  # x startup chunks (2 ko / 256KB each)
WCH = 4  # w1/v1 ff=0 startup chunks (8 ko / 256KB each)

last_exec_time_ns = None
_cache = {}


def _build():
    nc = bacc.Bacc("TRN2", target_bir_lowering=False, debug=False)
    xT_d = nc.dram_tensor("xT", [P, KO * M], BF16, kind="ExternalInput").ap()
    w1_d = nc.dram_tensor("w1t", [FFO, P, KO * P], BF16, kind="ExternalInput").ap()
    v1_d = nc.dram_tensor("v1t", [FFO, P, KO * P], BF16, kind="ExternalInput").ap()
    w2_d = nc.dram_tensor("w2t", [HHO, P, FFO * P], BF16, kind="ExternalInput").ap()
    out_d = nc.dram_tensor("outT", [HHO, P, M], FP32, kind="ExternalOutput").ap()

    with TileContext(nc) as tc:
        with (
            tc.tile_pool(name="hpool", bufs=1) as hpool,
            tc.tile_pool(name="w2fpool", bufs=1) as w2fpool,
            # Opened before the phase-1 PSUM pools so its banks never
            # collide with them: the first phase-2 matmul must not wait
            # for the last phase-1 silu/mul PSUM reads (WAR).
            tc.tile_pool(name="ps2", bufs=2, space="PSUM") as ps2,
        ):
            # hT[ff] tiles live here across both phases:
            # slice [:, ff*M:(ff+1)*M] holds hT rows ff*128..ff*128+127.
            hT = hpool.tile([P, FFO * M], BF16)
            # First W2PRE ff-chunks of w2[hh=0], filled during phase 1.
            w2_first = w2fpool.tile([P, W2PRE * P], BF16)

            with (
                tc.tile_pool(name="xpool", bufs=1) as xpool,
                tc.tile_pool(name="wpool", bufs=3) as wpool,
                tc.tile_pool(name="pspool", bufs=2, space="PSUM") as pspool,
                tc.tile_pool(name="actpool", bufs=2) as actpool,
                tc.tile_pool(name="warmps", bufs=1, space="PSUM") as warmps,
            ):
                # Warm the PE HAM clock gate while the DMA engines spin up
                # and the initial tiles stream in. Inputs are uninitialized
                # SBUF garbage (this kernel only ever runs on hardware —
                # no sim uninit checks) so the junk has no dependencies at
                # all and starts as soon as the queue drains the preamble.
                # An idle PE down-clocks (~2x) for ~3us on resume, so junk
                # also pads the DMA-bound stretches below.
                warmp = warmps.tile([P, 4 * P], FP32)
                for i in range(N_JUNK_BIG):
                    nc.tensor.matmul(
                        warmp[:], hT[:, :P], hT[:, : 4 * P],
                        start=(i == 0), stop=(i == N_JUNK_BIG - 1),
                    )
                for i in range(N_JUNK_SMALL):
                    nc.tensor.matmul(
                        warmp[:, :P], hT[:, :P], hT[:, :P],
                        start=(i == 0), stop=(i == N_JUNK_SMALL - 1),
                    )

                def _junk():
                    nc.tensor.matmul(
                        warmp[:, :P], hT[:, :P], hT[:, :P],
                        start=True, stop=True,
                    )

                # xT resident: xt[ki, ko*M+m] = x[m, ko*128+ki]. The first
                # w1/v1 tiles are DMA'd per-ko interleaved with x so the
                # ff=0 matmuls can chase the DMA stream.
                # Descriptor issue on the sync engine costs ~0.6us each, so
                # startup chunks start small (first matmul deps land early)
                # and coarsen, issued in the order the interleaved gate/up
                # matmuls consume them.
                xt = xpool.tile([P, KO * M], BF16)
                w1t0 = wpool.tile([P, KO * P], BF16, tag="w1")
                v1t0 = wpool.tile([P, KO * P], BF16, tag="v1")
                w1t1 = wpool.tile([P, KO * P], BF16, tag="w1")
                v1t1 = wpool.tile([P, KO * P], BF16, tag="v1")

                # All startup DMAs go on the single Sync HWDGE ring, in
                # exact consumption order: the ring drains FIFO, so the
                # stream never spends bandwidth on bytes the PE doesn't
                # need yet. Chunks are <=256KB so a consumer never waits
                # on more than ~0.7us of in-flight transfer — the coarse
                # 512KB x chunks were what the slow-ramp runs' 2.5-4us
                # stalls (and the HAM re-throttle they tripped) waited
                # on. The junk prefix still defers the first real matmul
                # to ~17.5us: starting earlier ran real matmuls inside
                # the cold 1.2GHz HAM window and lost every time.
                XS = KO * M // XCH  # x chunk cols (2 ko)
                WS = KO * P // WCH  # w chunk cols (8 ko)

                def _dx(c):
                    nc.sync.dma_start(
                        out=xt[:, c * XS : (c + 1) * XS],
                        in_=xT_d[:, c * XS : (c + 1) * XS],
                    )

                def _dw(c):
                    nc.sync.dma_start(
                        out=w1t0[:, c * WS : (c + 1) * WS],
                        in_=w1_d[0][:, c * WS : (c + 1) * WS],
                    )
                    nc.sync.dma_start(
                        out=v1t0[:, c * WS : (c + 1) * WS],
                        in_=v1_d[0][:, c * WS : (c + 1) * WS],
                    )

                def _dw1(t, s, k0, k1):
                    nc.sync.dma_start(
                        out=t[:, k0 * P : k1 * P], in_=s[1][:, k0 * P : k1 * P]
                    )

                # x chunk c covers ko 2c..2c+1; w/v chunk c covers ko
                # 8c..8c+7. Issue each just before its consumers.
                _dx(0); _dw(0); _dx(1); _dx(2); _dx(3)
                _dx(4); _dw(1); _dx(5); _dx(6); _dx(7)
                _dx(8); _dw(2); _dx(9); _dx(10); _dx(11)
                _dx(12); _dw(3); _dx(13); _dx(14); _dx(15)
                # w1/v1 for ff=1 in halves: the first halves land ~2us
                # earlier than a single full-tile transfer, closing the
                # ff=0 -> ff=1 handoff stall.
                _dw1(w1t1, w1_d, 0, KO // 2)
                _dw1(v1t1, v1_d, 0, KO // 2)
                _dw1(w1t1, w1_d, KO // 2, KO)
                _dw1(v1t1, v1_d, KO // 2, KO)

                for ff in range(FFO):
                    # w tile: [ki, ko*128+f] = w1[ko*128+ki, ff*128+f]
                    if ff == 0:
                        w1_tile, v1_tile = w1t0, v1t0
                    elif ff == 1:
                        w1_tile, v1_tile = w1t1, v1t1
                    else:
                        w1_tile = wpool.tile([P, KO * P], BF16, tag="w1")
                        v1_tile = wpool.tile([P, KO * P], BF16, tag="v1")
                        nc.sync.dma_start(out=w1_tile[:], in_=w1_d[ff])
                        nc.sync.dma_start(out=v1_tile[:], in_=v1_d[ff])
                        # Pace the w2[0] prefetch behind this iteration's
                        # weight fetches: 8 ff-chunks per iteration.
                        c = ff - 4
                        lo, hi = c * 8 * P, min((c + 1) * 8, W2PRE) * P
                        if 0 <= c and lo < W2PRE * P:
                            nc.sync.dma_start(
                                out=w2_first[:, lo:hi],
                                in_=w2_d[0][:, lo:hi],
                            )

                    pg = pspool.tile([P, M], FP32, tag="pg")
                    pu = pspool.tile([P, M], FP32, tag="pu")
                    for ko in range(KO):
                        nc.tensor.matmul(
                            pg[:],
                            w1_tile[:, ko * P : (ko + 1) * P],
                            xt[:, ko * M : (ko + 1) * M],
                            start=(ko == 0),
                            stop=(ko == KO - 1),
                        )
                        nc.tensor.matmul(
                            pu[:],
                            v1_tile[:, ko * P : (ko + 1) * P],
                            xt[:, ko * M : (ko + 1) * M],
                            start=(ko == 0),
                            stop=(ko == KO - 1),
                        )
                        # A little junk through the first kos pads the
                        # earliest chunk raggedness. Beyond that it's
                        # useless in both regimes (trace-verified): on
                        # delivery-bound 2.4GHz runs the stalls happen
                        # with or without it, and on PE-paced 2.0GHz runs
                        # each junk adds its full 53-64ns to ff=0.
                        if ff == 0 and ko < 8:
                            _junk()
                    sg = actpool.tile([P, M], FP32, tag="sg")
                    nc.scalar.activation(
                        sg[:], pg[:], mybir.ActivationFunctionType.Silu
                    )
                    nc.vector.tensor_mul(
                        out=hT[:, ff * M : (ff + 1) * M], in0=sg[:], in1=pu[:]
                    )

            with (
                tc.tile_pool(name="w2tailp", bufs=1) as w2tailp,
                tc.tile_pool(name="w2pool", bufs=2) as w2pool,
                tc.tile_pool(name="opool", bufs=2) as opool,
            ):
                # Chunks of w2[hh=0] that didn't fit next to the phase-1
                # pools; fetched first thing in phase 2. Split into 4
                # descriptors so the first chunks' completion doesn't
                # wait on the whole transfer (hh=0 consumes chunk c at
                # ~c*216ns into phase 2).
                NT = FFO - W2PRE
                w2_tail = w2tailp.tile([P, NT * P], BF16)
                for t0 in range(0, NT, (NT + 3) // 4):
                    t1 = min(t0 + (NT + 3) // 4, NT)
                    nc.sync.dma_start(
                        out=w2_tail[:, t0 * P : t1 * P],
                        in_=w2_d[0][:, (W2PRE + t0) * P : (W2PRE + t1) * P],
                    )
                half = M // 2
                for hh in range(HHO):
                    # w2 tile: [ki, ffo*128+f] = w2[hh*128+f, ffo*128+ki]
                    if hh == 0:
                        w2_tile = None
                    else:
                        w2_tile = w2pool.tile([P, FFO * P], BF16, tag="w2")
                        nc.sync.dma_start(out=w2_tile[:], in_=w2_d[hh])

                    def _w2src(ff):
                        if hh == 0:
                            if ff < W2PRE:
                                return w2_first[:, ff * P : (ff + 1) * P]
                            c = ff - W2PRE
                            return w2_tail[:, c * P : (c + 1) * P]
                        return w2_tile[:, ff * P : (ff + 1) * P]

                    if hh < HHO - 1:
                        pd = ps2.tile([P, M], FP32, tag="pd")
                        for ff in range(FFO):
                            nc.tensor.matmul(
                                pd[:],
                                _w2src(ff),
                                hT[:, ff * M : (ff + 1) * M],
                                start=(ff == 0),
                                stop=(ff == FFO - 1),
                            )
                        ot = opool.tile([P, M], FP32, tag="ot")
                        nc.vector.tensor_copy(out=ot[:], in_=pd[:])
                        nc.sync.dma_start(out=out_d[hh], in_=ot[:])
                    else:
                        # Last tile: two half-width accumulation chains so
                        # the first half's copy+DMA hides under the second
                        # half's matmuls; only ~1.9us of copy+DMA remains
                        # exposed after the final matmul. Full-size [P, M]
                        # psum tiles keep the two chains in separate banks
                        # (PE-write + DVE-read same bank is fatal).
                        for c in range(2):
                            lo = c * half
                            pd = ps2.tile([P, M], FP32, tag="pd")
                            for ff in range(FFO):
                                nc.tensor.matmul(
                                    pd[:, :half],
                                    _w2src(ff),
                                    hT[:, ff * M + lo : ff * M + lo + half],
                                    start=(ff == 0),
                                    stop=(ff == FFO - 1),
                                )
                            ot = opool.tile([P, M], FP32, tag="ot")
                            if c == 0:
                                # Fully hidden under chain 1's matmuls.
                                nc.vector.tensor_copy(
                                    out=ot[:, :half], in_=pd[:, :half]
                                )
                                nc.sync.dma_start(
                                    out=out_d[hh][:, lo : lo + half],
                                    in_=ot[:, :half],
                                )
                            else:
                                # Exposed tail: quarter-split so copy2
                                # overlaps DMA1's descriptor issue and
                                # the final completion wait covers only
                                # 64KB instead of 128KB.
                                q = half // 2
                                nc.vector.tensor_copy(
                                    out=ot[:, :q], in_=pd[:, :q]
                                )
                                nc.sync.dma_start(
                                    out=out_d[hh][:, lo : lo + q],
                                    in_=ot[:, :q],
                                )
                                nc.vector.tensor_copy(
                                    out=ot[:, q:half], in_=pd[:, q:half]
                                )
                                nc.sync.dma_start(
                                    out=out_d[hh][:, lo + q : lo + half],
                                    in_=ot[:, q:half],
                                )
    nc.compile()
    return nc


def _prep_weights(expert_w1, expert_v1, expert_w2):
    bf = ml_dtypes.bfloat16
    # w1t[ffo, ki, ko*P+f] = w1[ko*P+ki, ffo*P+f]
    w1t = np.ascontiguousarray(
        expert_w1.reshape(KO, P, FFO, P).transpose(2, 1, 0, 3).reshape(FFO, P, KO * P)
    ).astype(bf)
    v1t = np.ascontiguousarray(
        expert_v1.reshape(KO, P, FFO, P).transpose(2, 1, 0, 3).reshape(FFO, P, KO * P)
    ).astype(bf)
    # w2t[hho, ki, ffo*P+f] = w2[hho*P+f, ffo*P+ki]
    w2t = np.ascontiguousarray(
        expert_w2.reshape(HHO, P, FFO, P).transpose(0, 3, 2, 1).reshape(HHO, P, FFO * P)
    ).astype(bf)
    return w1t, v1t, w2t


def kernel(x, expert_w1, expert_v1, expert_w2):
    global last_exec_time_ns
    x = np.asarray(x, dtype=np.float32)
    w1t, v1t, w2t = _prep_weights(
        np.asarray(expert_w1, np.float32),
        np.asarray(expert_v1, np.float32),
        np.asarray(expert_w2, np.float32),
    )

    bf = ml_dtypes.bfloat16
    in_maps = []
    for c in range(NCORES):
        xs = x[c * M : (c + 1) * M]  # [M, H]
        # xt[ki, ko*M+m] = xs[m, ko*P+ki]
        xt = np.ascontiguousarray(
            xs.reshape(M, KO, P).transpose(2, 1, 0).reshape(P, KO * M)
        ).astype(bf)
        in_maps.append({"xT": xt, "w1t": w1t, "v1t": v1t, "w2t": w2t})

    if "nc" not in _cache:
        _cache["nc"] = _build()
    nc = _cache["nc"]

    trace = os.environ.get("KERNEL_TRACE", "") == "1"
    if trace:
        _install_ntff_hook()
    res = None
    for attempt in range(3):
        try:
            res = bass_utils.run_bass_kernel_spmd(
                nc, in_maps, core_ids=list(range(NCORES)), trace=trace
            )
            break
        except Exception:
            # The tunneled device occasionally reports a transient
            # "unrecoverable" state left over from a prior session; it
            # clears on retry.
            if attempt == 2:
                raise
            import time

            time.sleep(20)
    last_exec_time_ns = res.exec_time_ns

    # results[c]["outT"] is downT for core c: [HHO, P, M] with
    # outT[hh, j, m] = down[c*M+m, hh*P+j]
    out = np.empty((T, H), np.float32)
    for c in range(NCORES):
        o = res.results[c]["outT"].reshape(H, M)
        out[c * M : (c + 1) * M] = o.T
    return out


def _install_ntff_hook():
    """Wire the axon NTFF profile hook this image's antenv lacks."""
    import importlib.util
    import sys
    import types

    if "antenv.axon_hooks" in sys.modules:
        return
    so_path = "/opt/axon/libaxon_pjrt.so"
    boot = "/root/.axon_site/trn_agent_boot/trn_boot.py"
    if not (os.path.exists(so_path) and os.path.exists(boot)):
        return
    spec = importlib.util.spec_from_file_location("trn_boot_local", boot)
    trn_boot = importlib.util.module_from_spec(spec)
    spec.loader.exec_module(trn_boot)
    hook = trn_boot._ntff_profile_via_ctypes(so_path)
    m = types.ModuleType("antenv.axon_hooks")
    m.get_axon_ntff_profile_hook = lambda: hook
    m.set_axon_ntff_profile_hook = lambda h: None
    sys.modules["antenv.axon_hooks"] = m



# revision 30
# speedup vs baseline: 1.0031x; 1.0005x over previous
"""SwiGLU expert FFN (DbrxExpertGLU) on 8 Trainium2 NeuronCores.

reference: down = (silu(x @ w1) * (x @ v1)) @ w2.T
  x [T=4096, H=4096], w1/v1/w2 [H=4096, F=14336], all fp32.

Strategy: token-parallel — shard T across the 8 cores (512 rows each),
replicate weights; no collectives. Each core computes everything
transposed so both matmul stages contract over the partition dim:

  phase 1:  gateT/upT [F, m] = w1T/v1T-tiles.T @ xT   (accumulate over H)
            hT = silu(gateT) * upT                     (elementwise, bf16)
  phase 2:  downT [H, m]     = w2T-tiles.T @ hT        (accumulate over F)

Matmuls run in bf16 (PE peak 78.6 TF/s) with fp32 PSUM accumulation.
Host pre-casts weights to bf16 and retiles them so every weight DMA is
a contiguous [128, *] full-partition transfer.

Schedule notes (from neuron-profile trace analysis):
  - PE cadence is 216 ns per [128x128]x[128,512] matmul at 2.4 GHz
    (259 ns when the chip's power throttle caps the PE at 2.0 GHz —
    a sticky, run-to-run state that explains ~1.2x exec-time swings),
    so the bf16 matmul floor is 2.29-2.78 ms/core depending on clock;
    everything else is overlap. fp8/int8 cannot beat this: DoubleRow
    is e4m3/e5m2-only and their quantization noise (3.8-6.6% end to
    end) fails the 2e-2 gate.
  - Warmup junk matmuls are NOT gated on any DMA (first data lands
    ~10-13 us after the body gate) so the HAM clock ramp and the DMA
    ramp run concurrently; the prefix defers the first real matmul to
    ~17.5 us (2.4 GHz) — every earlier start measured worse: real
    matmuls land in the cold 1.2 GHz window and the erratic early
    DMA ramp (0.2-0.42 MB/us) adds a 2-5 us contiguous stall that
    trips a HAM re-throttle.
  - gate/up matmuls interleave per ko; x streams in 2-ko/256KB chunks
    and w1/v1 in 8-ko/256KB chunks on the single Sync HWDGE ring in
    exact consumption order (a Sync+Scalar two-ring split delivers
    out of order and starves the x stream).
  - w2[hh=0] is prefetched into a persistent pool during phase 1
    (paced 2 chunks per ff iteration so it never starves the w1/v1
    ring), eliminating the 15 us phase-1->phase-2 DMA gap.
  - The last hh runs as two N=256 accumulation chains so half the
    final copy+DMA hides under matmuls (exposed tail 5.3 -> 2.5 us).

Set KERNEL_TRACE=1 to capture an NTFF profile; the HW exec time lands
in kernel.last_exec_time_ns.
"""

import os

import numpy as np
import ml_dtypes

import concourse.mybir as mybir
from concourse import bacc, bass_utils
from concourse.tile import TileContext

T, H, F = 4096, 4096, 14336
NCORES = 8
M = T // NCORES  # 512 token rows per core
P = 128
KO = H // P  # 32  k-tiles (phase-1 contraction)
FFO = F // P  # 112 f-tiles
HHO = H // P  # 32  output-row tiles (phase-2)
BF16 = mybir.dt.bfloat16
FP32 = mybir.dt.float32

N_JUNK_BIG = 18  # 512-row warmup matmuls (PE clock ramp, DMA spin-up)
N_JUNK_SMALL = 40  # 128-row warmup matmuls (fine-grained bridge)
W2PRE = 44  # ff-chunks of w2[hh=0] prefetched during phase 1
XCH = 16  # x startup chunks (2 ko / 256KB each)
WCH = 4  # w1/v1 ff=0 startup chunks (8 ko / 256KB each)

last_exec_time_ns = None
_cache = {}


def _build():
    nc = bacc.Bacc("TRN2", target_bir_lowering=False, debug=False)
    xT_d = nc.dram_tensor("xT", [P, KO * M], BF16, kind="ExternalInput").ap()
    w1_d = nc.dram_tensor("w1t", [FFO, P, KO * P], BF16, kind="ExternalInput").ap()
    v1_d = nc.dram_tensor("v1t", [FFO, P, KO * P], BF16, kind="ExternalInput").ap()
    w2_d = nc.dram_tensor("w2t", [HHO, P, FFO * P], BF16, kind="ExternalInput").ap()
    out_d = nc.dram_tensor("outT", [HHO, P, M], FP32, kind="ExternalOutput").ap()

    with TileContext(nc) as tc:
        with (
            tc.tile_pool(name="hpool", bufs=1) as hpool,
            tc.tile_pool(name="w2fpool", bufs=1) as w2fpool,
            # Opened before the phase-1 PSUM pools so its banks never
            # collide with them: the first phase-2 matmul must not wait
            # for the last phase-1 silu/mul PSUM reads (WAR).
            tc.tile_pool(name="ps2", bufs=2, space="PSUM") as ps2,
        ):
            # hT[ff] tiles live here across both phases:
            # slice [:, ff*M:(ff+1)*M] holds hT rows ff*128..ff*128+127.
            hT = hpool.tile([P, FFO * M], BF16)
            # First W2PRE ff-chunks of w2[hh=0], filled during phase 1.
            w2_first = w2fpool.tile([P, W2PRE * P], BF16)

            with (
                tc.tile_pool(name="xpool", bufs=1) as xpool,
                tc.tile_pool(name="wpool", bufs=3) as wpool,
                tc.tile_pool(name="pspool", bufs=2, space="PSUM") as pspool,
                tc.tile_pool(name="actpool", bufs=2) as actpool,
                tc.tile_pool(name="warmps", bufs=1, space="PSUM") as warmps,
            ):
                # Warm the PE HAM clock gate while the DMA engines spin up
                # and the initial tiles stream in. Inputs are uninitialized
                # SBUF garbage (this kernel only ever runs on hardware —
                # no sim uninit checks) so the junk has no dependencies at
                # all and starts as soon as the queue drains the preamble.
                # An idle PE down-clocks (~2x) for ~3us on resume, so junk
                # also pads the DMA-bound stretches below.
                warmp = warmps.tile([P, 4 * P], FP32)
                for i in range(N_JUNK_BIG):
                    nc.tensor.matmul(
                        warmp[:], hT[:, :P], hT[:, : 4 * P],
                        start=(i == 0), stop=(i == N_JUNK_BIG - 1),
                    )
                for i in range(N_JUNK_SMALL):
                    nc.tensor.matmul(
                        warmp[:, :P], hT[:, :P], hT[:, :P],
                        start=(i == 0), stop=(i == N_JUNK_SMALL - 1),
                    )

                def _junk():
                    nc.tensor.matmul(
                        warmp[:, :P], hT[:, :P], hT[:, :P],
                        start=True, stop=True,
                    )

                # xT resident: xt[ki, ko*M+m] = x[m, ko*128+ki]. The first
                # w1/v1 tiles are DMA'd per-ko interleaved with x so the
                # ff=0 matmuls can chase the DMA stream.
                # Descriptor issue on the sync engine costs ~0.6us each, so
                # startup chunks start small (first matmul deps land early)
                # and coarsen, issued in the order the interleaved gate/up
                # matmuls consume them.
                xt = xpool.tile([P, KO * M], BF16)
                w1t0 = wpool.tile([P, KO * P], BF16, tag="w1")
                v1t0 = wpool.tile([P, KO * P], BF16, tag="v1")
                w1t1 = wpool.tile([P, KO * P], BF16, tag="w1")
                v1t1 = wpool.tile([P, KO * P], BF16, tag="v1")

                # All startup DMAs go on the single Sync HWDGE ring, in
                # exact consumption order: the ring drains FIFO, so the
                # stream never spends bandwidth on bytes the PE doesn't
                # need yet. Chunks are <=256KB so a consumer never waits
                # on more than ~0.7us of in-flight transfer — the coarse
                # 512KB x chunks were what the slow-ramp runs' 2.5-4us
                # stalls (and the HAM re-throttle they tripped) waited
                # on. The junk prefix still defers the first real matmul
                # to ~17.5us: starting earlier ran real matmuls inside
                # the cold 1.2GHz HAM window and lost every time.
                XS = KO * M // XCH  # x chunk cols (2 ko)
                WS = KO * P // WCH  # w chunk cols (8 ko)

                def _dx(c):
                    nc.sync.dma_start(
                        out=xt[:, c * XS : (c + 1) * XS],
                        in_=xT_d[:, c * XS : (c + 1) * XS],
                    )

                def _dw(c):
                    nc.sync.dma_start(
                        out=w1t0[:, c * WS : (c + 1) * WS],
                        in_=w1_d[0][:, c * WS : (c + 1) * WS],
                    )
                    nc.sync.dma_start(
                        out=v1t0[:, c * WS : (c + 1) * WS],
                        in_=v1_d[0][:, c * WS : (c + 1) * WS],
                    )

                def _dw1(t, s, k0, k1):
                    nc.sync.dma_start(
                        out=t[:, k0 * P : k1 * P], in_=s[1][:, k0 * P : k1 * P]
                    )

                # x chunk c covers ko 2c..2c+1; w/v chunk c covers ko
                # 8c..8c+7. Issue each just before its consumers.
                _dx(0); _dw(0); _dx(1); _dx(2); _dx(3)
                _dx(4); _dw(1); _dx(5); _dx(6); _dx(7)
                _dx(8); _dw(2); _dx(9); _dx(10); _dx(11)
                _dx(12); _dw(3); _dx(13); _dx(14); _dx(15)
                # w1/v1 for ff=1 in halves: the first halves land ~2us
                # earlier than a single full-tile transfer, closing the
                # ff=0 -> ff=1 handoff stall.
                _dw1(w1t1, w1_d, 0, KO // 2)
                _dw1(v1t1, v1_d, 0, KO // 2)
                _dw1(w1t1, w1_d, KO // 2, KO)
                _dw1(v1t1, v1_d, KO // 2, KO)

                for ff in range(FFO):
                    # w tile: [ki, ko*128+f] = w1[ko*128+ki, ff*128+f]
                    if ff == 0:
                        w1_tile, v1_tile = w1t0, v1t0
                    elif ff == 1:
                        w1_tile, v1_tile = w1t1, v1t1
                    else:
                        w1_tile = wpool.tile([P, KO * P], BF16, tag="w1")
                        v1_tile = wpool.tile([P, KO * P], BF16, tag="v1")
                        nc.sync.dma_start(out=w1_tile[:], in_=w1_d[ff])
                        nc.sync.dma_start(out=v1_tile[:], in_=v1_d[ff])
                        # Pace the w2[0] prefetch behind this iteration's
                        # weight fetches: 8 ff-chunks per iteration.
                        c = ff - 4
                        lo, hi = c * 8 * P, min((c + 1) * 8, W2PRE) * P
                        if 0 <= c and lo < W2PRE * P:
                            nc.sync.dma_start(
                                out=w2_first[:, lo:hi],
                                in_=w2_d[0][:, lo:hi],
                            )

                    pg = pspool.tile([P, M], FP32, tag="pg")
                    pu = pspool.tile([P, M], FP32, tag="pu")
                    for ko in range(KO):
                        nc.tensor.matmul(
                            pg[:],
                            w1_tile[:, ko * P : (ko + 1) * P],
                            xt[:, ko * M : (ko + 1) * M],
                            start=(ko == 0),
                            stop=(ko == KO - 1),
                        )
                        nc.tensor.matmul(
                            pu[:],
                            v1_tile[:, ko * P : (ko + 1) * P],
                            xt[:, ko * M : (ko + 1) * M],
                            start=(ko == 0),
                            stop=(ko == KO - 1),
                        )
                        # No in-loop junk: trace-verified useless in both
                        # regimes — on delivery-bound 2.4GHz runs stalls
                        # happen with or without it (strict-FIFO queues
                        # can't fill an unpredicted delivery hole), and
                        # on PE-paced 2.0GHz runs each junk adds its full
                        # 53-64ns to ff=0. Startup stalls land at
                        # ko~10-20, never in the early kos it padded.
                    sg = actpool.tile([P, M], FP32, tag="sg")
                    nc.scalar.activation(
                        sg[:], pg[:], mybir.ActivationFunctionType.Silu
                    )
                    nc.vector.tensor_mul(
                        out=hT[:, ff * M : (ff + 1) * M], in0=sg[:], in1=pu[:]
                    )

            with (
                tc.tile_pool(name="w2tailp", bufs=1) as w2tailp,
                tc.tile_pool(name="w2pool", bufs=2) as w2pool,
                tc.tile_pool(name="opool", bufs=2) as opool,
            ):
                # Chunks of w2[hh=0] that didn't fit next to the phase-1
                # pools; fetched first thing in phase 2. Split into 4
                # descriptors so the first chunks' completion doesn't
                # wait on the whole transfer (hh=0 consumes chunk c at
                # ~c*216ns into phase 2).
                NT = FFO - W2PRE
                w2_tail = w2tailp.tile([P, NT * P], BF16)
                for t0 in range(0, NT, (NT + 3) // 4):
                    t1 = min(t0 + (NT + 3) // 4, NT)
                    nc.sync.dma_start(
                        out=w2_tail[:, t0 * P : t1 * P],
                        in_=w2_d[0][:, (W2PRE + t0) * P : (W2PRE + t1) * P],
                    )
                half = M // 2
                for hh in range(HHO):
                    # w2 tile: [ki, ffo*128+f] = w2[hh*128+f, ffo*128+ki]
                    if hh == 0:
                        w2_tile = None
                    else:
                        w2_tile = w2pool.tile([P, FFO * P], BF16, tag="w2")
                        nc.sync.dma_start(out=w2_tile[:], in_=w2_d[hh])

                    def _w2src(ff):
                        if hh == 0:
                            if ff < W2PRE:
                                return w2_first[:, ff * P : (ff + 1) * P]
                            c = ff - W2PRE
                            return w2_tail[:, c * P : (c + 1) * P]
                        return w2_tile[:, ff * P : (ff + 1) * P]

                    if hh < HHO - 1:
                        pd = ps2.tile([P, M], FP32, tag="pd")
                        for ff in range(FFO):
                            nc.tensor.matmul(
                                pd[:],
                                _w2src(ff),
                                hT[:, ff * M : (ff + 1) * M],
                                start=(ff == 0),
                                stop=(ff == FFO - 1),
                            )
                        ot = opool.tile([P, M], FP32, tag="ot")
                        nc.vector.tensor_copy(out=ot[:], in_=pd[:])
                        nc.sync.dma_start(out=out_d[hh], in_=ot[:])
                    else:
                        # Last tile: two half-width accumulation chains so
                        # the first half's copy+DMA hides under the second
                        # half's matmuls; only ~1.9us of copy+DMA remains
                        # exposed after the final matmul. Full-size [P, M]
                        # psum tiles keep the two chains in separate banks
                        # (PE-write + DVE-read same bank is fatal).
                        for c in range(2):
                            lo = c * half
                            pd = ps2.tile([P, M], FP32, tag="pd")
                            for ff in range(FFO):
                                nc.tensor.matmul(
                                    pd[:, :half],
                                    _w2src(ff),
                                    hT[:, ff * M + lo : ff * M + lo + half],
                                    start=(ff == 0),
                                    stop=(ff == FFO - 1),
                                )
                            ot = opool.tile([P, M], FP32, tag="ot")
                            # One copy + one DMA per chain. (A further
                            # quarter-split of the exposed chain was
                            # tried: the second DMA's descriptor issue
                            # serializes behind the first via DMA-lane
                            # sem reuse and measured ~1us WORSE.)
                            nc.vector.tensor_copy(
                                out=ot[:, :half], in_=pd[:, :half]
                            )
                            nc.sync.dma_start(
                                out=out_d[hh][:, lo : lo + half],
                                in_=ot[:, :half],
                            )
    nc.compile()
    return nc


def _prep_weights(expert_w1, expert_v1, expert_w2):
    bf = ml_dtypes.bfloat16
    # w1t[ffo, ki, ko*P+f] = w1[ko*P+ki, ffo*P+f]
    w1t = np.ascontiguousarray(
        expert_w1.reshape(KO, P, FFO, P).transpose(2, 1, 0, 3).reshape(FFO, P, KO * P)
    ).astype(bf)
    v1t = np.ascontiguousarray(
        expert_v1.reshape(KO, P, FFO, P).transpose(2, 1, 0, 3).reshape(FFO, P, KO * P)
    ).astype(bf)
    # w2t[hho, ki, ffo*P+f] = w2[hho*P+f, ffo*P+ki]
    w2t = np.ascontiguousarray(
        expert_w2.reshape(HHO, P, FFO, P).transpose(0, 3, 2, 1).reshape(HHO, P, FFO * P)
    ).astype(bf)
    return w1t, v1t, w2t


def kernel(x, expert_w1, expert_v1, expert_w2):
    global last_exec_time_ns
    x = np.asarray(x, dtype=np.float32)
    w1t, v1t, w2t = _prep_weights(
        np.asarray(expert_w1, np.float32),
        np.asarray(expert_v1, np.float32),
        np.asarray(expert_w2, np.float32),
    )

    bf = ml_dtypes.bfloat16
    in_maps = []
    for c in range(NCORES):
        xs = x[c * M : (c + 1) * M]  # [M, H]
        # xt[ki, ko*M+m] = xs[m, ko*P+ki]
        xt = np.ascontiguousarray(
            xs.reshape(M, KO, P).transpose(2, 1, 0).reshape(P, KO * M)
        ).astype(bf)
        in_maps.append({"xT": xt, "w1t": w1t, "v1t": v1t, "w2t": w2t})

    if "nc" not in _cache:
        _cache["nc"] = _build()
    nc = _cache["nc"]

    trace = os.environ.get("KERNEL_TRACE", "") == "1"
    if trace:
        _install_ntff_hook()
    res = None
    for attempt in range(3):
        try:
            res = bass_utils.run_bass_kernel_spmd(
                nc, in_maps, core_ids=list(range(NCORES)), trace=trace
            )
            break
        except Exception:
            # The tunneled device occasionally reports a transient
            # "unrecoverable" state left over from a prior session; it
            # clears on retry.
            if attempt == 2:
                raise
            import time

            time.sleep(20)
    last_exec_time_ns = res.exec_time_ns

    # results[c]["outT"] is downT for core c: [HHO, P, M] with
    # outT[hh, j, m] = down[c*M+m, hh*P+j]
    out = np.empty((T, H), np.float32)
    for c in range(NCORES):
        o = res.results[c]["outT"].reshape(H, M)
        out[c * M : (c + 1) * M] = o.T
    return out


def _install_ntff_hook():
    """Wire the axon NTFF profile hook this image's antenv lacks."""
    import importlib.util
    import sys
    import types

    if "antenv.axon_hooks" in sys.modules:
        return
    so_path = "/opt/axon/libaxon_pjrt.so"
    boot = "/root/.axon_site/trn_agent_boot/trn_boot.py"
    if not (os.path.exists(so_path) and os.path.exists(boot)):
        return
    spec = importlib.util.spec_from_file_location("trn_boot_local", boot)
    trn_boot = importlib.util.module_from_spec(spec)
    spec.loader.exec_module(trn_boot)
    hook = trn_boot._ntff_profile_via_ctypes(so_path)
    m = types.ModuleType("antenv.axon_hooks")
    m.get_axon_ntff_profile_hook = lambda: hook
    m.set_axon_ntff_profile_hook = lambda h: None
    sys.modules["antenv.axon_hooks"] = m

